# revision 34
# baseline (speedup 1.0000x reference)
"""Condensation loss (Tiger) on 8 Trainium2 NeuronCores.

Strategy (per sharding hint): shard the hit dimension N across 8 cores,
replicate the K-1 condensation points, assemble the scalar loss on host.

Math restructure vs the baseline kernel: the repulsive term
  v_rep = sum_{n,k} q_n q_k (1 - dist_nk) [dist_nk < 1][~att]
is nonzero only for pairs with d2 < 1.  The device computes the full
N x K d2 matrix on the PE (bf16 inputs, fp32 PSUM) and reduces each row
to a tiny *detector* output instead of evaluating sqrt/min per element:
  - DVE lane:  tensor_reduce(min) -> rowmin of d2
  - ACT lane:  activation(Relu, scale=-1, bias=4) + accum_out
               -> rowsum of relu(4 - d2)
A row can contain a d2 < 1 pair only if its detector fires (bf16 input
rounding shifts d2 by well under the 2.5 flag margin; a guard falls back
to flagging everything for out-of-range inputs).  The host recomputes
flagged rows exactly in fp64 (~1k rows: the condensation points
themselves plus hit 0 for empty objects).  v_att (O(N*D)), l_noise and
l_coward are exact on host in fp64.

Device layout per core: 6400 padded hits = 50 row-tiles of 128.  Each
tile's d2 [128, 1024] lives in one 2-bank PSUM tile; 4 such buffers fill
all 8 banks so each drain engine stays independently double-buffered and
the matmul bursts hide entirely.  Even tiles compute at PE array rows
0:34, odd tiles at 64:98 (tile_position row-packing) so consecutive
tiles' LDWEIGHTS/MATMULs overlap.  Tiles are split ~26/24 between the
DVE and ACT drain lanes (measured ~1.2/1.3 us per [128,1024] fp32 PSUM
drain); both engines run gap-free, which is the 1 elem/cycle/lane fp32
PSUM-read floor of TRN2 (gpsimd and DMA have no PSUM port).
"""

import os
import numpy as np
import ml_dtypes

# ---------------- geometry (hardcoded per the task contract) ----------------
N_HITS = 50000
D_EMB = 32
N_CLUSTERS = 1024          # ids 0..1023; objects are 1..1023
N_OBJ = N_CLUSTERS - 1     # 1023
KP = 1024                  # padded object columns (col j = object j+1; col 1023 dummy)
NCORES = 8
N_PER = N_HITS // NCORES   # 6250
NP = 6400                  # padded rows per core = 50*128
NT = NP // 128             # 50 row tiles
NPAIR = NT // 2            # 25 tile pairs (xt packing unit)
CDIM = D_EMB + 2           # contraction: [x(32), r2, 1]
THR = 4.0                  # detector threshold on d2 (flag margin vs dist<1)
DVE_COST = 1224.0          # ns per tile drain on DVE (measured)
ACT_COST = 1300.0          # ns per tile drain on ACT (measured, incl READ_ACC)

Q_MIN = 0.01
PT_THLD = 0.9
MAX_ETA = 4.0
LW_REP = 1.0
LW_NOISE = 0.1
LW_COWARD = 0.1
EPS = 1e-9

_BF16 = ml_dtypes.bfloat16

_STATE = {}


def _tile_split():
    """Greedy least-loaded assignment of the 50 tile-drains to DVE/ACT."""
    dve, act = [], []
    lv = la = 0.0
    for t in range(NT):
        if lv + DVE_COST <= la + ACT_COST:
            dve.append(t)
            lv += DVE_COST
        else:
            act.append(t)
            la += ACT_COST
    return dve, act


# ---------------- device module ----------------
def _build_module():
    import concourse.bacc as bacc
    import concourse.mybir as mybir
    import concourse.tile as tile
    from contextlib import ExitStack

    dve_tiles, act_tiles = _tile_split()
    nv, na = len(dve_tiles), len(act_tiles)
    lane = {}
    for i, t in enumerate(dve_tiles):
        lane[t] = ("V", i)
    for i, t in enumerate(act_tiles):
        lane[t] = ("A", i)

    nc = bacc.Bacc("TRN2", target_bir_lowering=False, debug=False,
                   num_devices=NCORES)
    dt = mybir.dt

    xt_d = nc.dram_tensor("xt", [128, NPAIR * 128], dt.bfloat16,
                          kind="ExternalInput").ap()
    xkt_d = nc.dram_tensor("xkt", [128, KP], dt.bfloat16,
                           kind="ExternalInput").ap()
    detv_d = nc.dram_tensor("detv_out", [128, nv], dt.float32,
                            kind="ExternalOutput").ap()
    deta_d = nc.dram_tensor("deta_out", [128, na], dt.float32,
                            kind="ExternalOutput").ap()

    with tile.TileContext(nc) as tc, ExitStack() as ctx:
        consts = ctx.enter_context(tc.tile_pool(name="consts", bufs=1))
        scra_p = ctx.enter_context(tc.tile_pool(name="scra", bufs=2))
        psum = ctx.enter_context(tc.tile_pool(name="psum", bufs=4, space="PSUM"))

        # full 128-partition DMAs (partition parallelism sets DMA bandwidth);
        # xkt halves + first xt chunk split across the two DGE queues so the
        # first matmuls' operands land as early as possible
        xkt_sb = consts.tile([128, KP], dt.bfloat16)
        xt_sb = consts.tile([128, NPAIR * 128], dt.bfloat16)
        nc.scalar.dma_start(out=xkt_sb[:, 0:256], in_=xkt_d[:, 0:256])
        nc.sync.dma_start(out=xkt_sb[:, 256:512], in_=xkt_d[:, 256:512])
        nc.sync.dma_start(out=xt_sb[:, 0:256], in_=xt_d[:, 0:256])
        nc.scalar.dma_start(out=xkt_sb[:, 512:1024], in_=xkt_d[:, 512:1024])
        edges = [256, 896, 1664, 2432, NPAIR * 128]
        for a, b in zip(edges[:-1], edges[1:]):
            nc.sync.dma_start(out=xt_sb[:, a:b], in_=xt_d[:, a:b])
        thrb_sb = consts.tile([128, 1], dt.float32)
        nc.gpsimd.memset(thrb_sb, THR)
        detv_sb = consts.tile([128, nv], dt.float32)
        deta_sb = consts.tile([128, na], dt.float32)

        for t in range(NT):
            # even tiles live at PE array rows 0:34, odd tiles at 64:98 —
            # consecutive tiles' LDWEIGHTS/MATMULs overlap (per-subarray
            # concurrency), and the 4 psum buffers keep both drain engines
            # independently double-buffered.
            p, base = t // 2, (0 if t % 2 == 0 else 64)
            ps = psum.tile([128, 1024], dt.float32, tag="d2")
            lhs = xt_sb[base:base + CDIM, p * 128:(p + 1) * 128]
            nc.tensor.matmul(ps[:, 0:512], lhs, xkt_sb[base:base + CDIM, 0:512],
                             start=True, stop=True, tile_position=(base, 0))
            nc.tensor.matmul(ps[:, 512:1024], lhs,
                             xkt_sb[base:base + CDIM, 512:1024],
                             start=True, stop=True, tile_position=(base, 0))
            which, idx = lane[t]
            if which == "V":
                nc.vector.tensor_reduce(detv_sb[:, idx:idx + 1], ps,
                                        axis=mybir.AxisListType.X,
                                        op=mybir.AluOpType.min)
            else:
                scr = scra_p.tile([128, 1024], dt.bfloat16, tag="scra")
                nc.scalar.activation(
                    scr, ps, mybir.ActivationFunctionType.Relu,
                    bias=thrb_sb, scale=-1.0,
                    accum_out=deta_sb[:, idx:idx + 1])

        # drain the finished halves of the det outputs early; final halves
        # go out on both DGE queues in parallel
        nc.sync.dma_start(out=detv_d[:, 0:nv // 2], in_=detv_sb[:, 0:nv // 2])
        nc.scalar.dma_start(out=deta_d[:, 0:na // 2], in_=deta_sb[:, 0:na // 2])
        nc.sync.dma_start(out=detv_d[:, nv // 2:], in_=detv_sb[:, nv // 2:])
        nc.scalar.dma_start(out=deta_d[:, na // 2:], in_=deta_sb[:, na // 2:])

    nc.compile()
    return nc


def _get_module():
    if "nc" not in _STATE:
        _STATE["nc"] = _build_module()
    return _STATE["nc"]


# ---------------- host prep ----------------
def _prep(beta, x, pt, eta, reconstructable, cluster_ids):
    f32 = np.float32
    f64 = np.float64
    beta = np.asarray(beta, f32)
    x = np.ascontiguousarray(np.asarray(x, f32))
    pt = np.asarray(pt, f32)
    eta = np.asarray(eta, f32)
    recon = np.asarray(reconstructable)
    cid = np.asarray(cluster_ids).astype(np.int64)

    # alpha selection in fp32 to match the reference's argmax semantics
    q32 = (np.arctanh(np.clip(beta, 0.0, 1.0 - 1e-4)) ** 2 + Q_MIN).astype(f32)
    hit_ok = (recon > 0) & (pt > PT_THLD) & (np.abs(eta) < MAX_ETA)
    cid_eff = np.where(hit_ok, cid, 0)
    best = np.zeros(N_CLUSTERS, f32)
    np.maximum.at(best, cid_eff, q32)
    idx = np.full(N_CLUSTERS, N_HITS, np.int64)
    ismax = (q32 == best[cid_eff]) & (cid_eff > 0)
    np.minimum.at(idx, cid_eff[ismax], np.nonzero(ismax)[0])
    alphas = np.where(idx[1:] < N_HITS, idx[1:], 0)      # [1023]

    # device operands: bf16-quantized hits + condensation points
    xq = x.astype(_BF16)                                 # [N, 32]
    xqf = xq.astype(f32)
    r2q = np.einsum('nd,nd->n', xqf, xqf).astype(f32)
    r2b = r2q.astype(_BF16)

    X34 = np.zeros((NCORES * NP, CDIM), f32)
    real = np.zeros(NCORES * NP, bool)
    for c in range(NCORES):
        real[c * NP:c * NP + N_PER] = True
    X34[real, :D_EMB] = xqf
    X34[real, D_EMB] = r2b.astype(f32)
    X34[:, D_EMB + 1] = 1.0
    X34 = X34.astype(_BF16)

    Y34 = np.zeros((KP, CDIM), f32)
    Y34[:N_OBJ, :D_EMB] = -2.0 * xqf[alphas]
    Y34[:N_OBJ, D_EMB] = 1.0
    Y34[:N_OBJ, D_EMB + 1] = r2b[alphas].astype(f32)
    Y34[N_OBJ] = 0.0
    Y34[N_OBJ, D_EMB] = 1.0
    Y34[N_OBJ, D_EMB + 1] = 1e4                          # dummy far column
    Y34 = Y34.astype(_BF16)
    xkt = np.zeros((128, KP), _BF16)
    xkt[0:CDIM] = Y34.T
    xkt[64:64 + CDIM] = Y34.T

    in_maps = []
    for c in range(NCORES):
        A = X34[c * NP:(c + 1) * NP].reshape(NT, 128, CDIM).transpose(0, 2, 1)
        xt_c = np.zeros((128, NPAIR * 128), _BF16)
        xt_c[0:CDIM] = A[0::2].transpose(1, 0, 2).reshape(CDIM, NPAIR * 128)
        xt_c[64:64 + CDIM] = A[1::2].transpose(1, 0, 2).reshape(
            CDIM, NPAIR * 128)
        in_maps.append({"xt": np.ascontiguousarray(xt_c), "xkt": xkt})

    aux = dict(q32=q32, hit_ok=hit_ok, cid=cid, beta=beta, x=x,
               alphas=alphas)
    return in_maps, aux


# ---------------- host finish ----------------
def _finish(results, aux):
    f64 = np.float64
    q32, alphas = aux["q32"], aux["alphas"]
    hit_ok, cid, beta, x = aux["hit_ok"], aux["cid"], aux["beta"], aux["x"]

    q = q32.astype(f64)
    x64 = x.astype(f64)
    xk64 = x64[alphas]                                   # [1023, 32]
    qk = q[alphas]

    dve_tiles, act_tiles = _tile_split()

    # ---- gather flagged hits from the detectors ----
    flagged = set()
    for c in range(NCORES):
        detv = np.asarray(results[c]["detv_out"], f64)   # [128, nv]
        deta = np.asarray(results[c]["deta_out"], f64)   # [128, na]
        fl = np.zeros((128, NT), bool)
        fl[:, dve_tiles] = detv < THR - 0.5
        fl[:, act_tiles] = deta > 0.45
        rr, tt = np.nonzero(fl)
        for r, t in zip(rr, tt):
            n = t * 128 + r
            if n < N_PER:
                flagged.add(c * N_PER + n)
    flagged = np.fromiter(sorted(flagged), dtype=np.int64,
                          count=len(flagged))

    # safety: the detector's bf16 error margin assumes moderate |x|; the
    # dominant term is the bf16 rounding of |x|^2, so bound that directly
    if (not np.isfinite(x).all()) or \
            float(np.einsum('nd,nd->n', x64, x64).max()) > 200.0:
        flagged = np.arange(N_HITS, dtype=np.int64)
    if os.environ.get("COND_KERNEL_DEBUG", "0") == "1":
        print(f"[kernel] flagged rows: {len(flagged)}")

    # ---- v_rep: exact fp64 over flagged rows only ----
    v_rep_num = 0.0
    if len(flagged):
        xf = x64[flagged]
        d2 = (np.einsum('nd,nd->n', xf, xf)[:, None]
              + np.einsum('kd,kd->k', xk64, xk64)[None, :]
              - 2.0 * (xf @ xk64.T))
        dist = np.sqrt(np.maximum(d2, 1e-12))
        att = (cid[flagged][:, None] == np.arange(1, N_CLUSTERS)[None, :]) \
            & hit_ok[flagged][:, None]
        rep = (~att) & (dist < 1.0)
        qw = q[flagged][:, None] * qk[None, :]
        v_rep_num = float(np.sum(qw * (1.0 - dist) * rep))

    # ---- v_att: exact fp64 on the attractive pairs ----
    att_idx = np.nonzero(hit_ok & (cid > 0))[0]
    kk = cid[att_idx] - 1
    diff = x64[att_idx] - xk64[kk]
    d2a = np.maximum(np.einsum('nd,nd->n', diff, diff), 1e-12)
    v_att_num = float(np.sum(q[att_idx] * qk[kk] * d2a))

    n_hits_oi = float(hit_ok.sum())
    norm_att = EPS + n_hits_oi - N_OBJ
    norm_rep = EPS + (N_OBJ - 1) * N_HITS
    v_att = v_att_num / norm_att
    v_rep = v_rep_num / norm_rep

    noise_mask = (cid <= 0)
    l_noise = float(beta[noise_mask].sum()) / max(float(noise_mask.sum()), 1.0)
    l_coward = float(np.mean(1.0 - beta[alphas]))

    total = v_att + LW_REP * v_rep + LW_NOISE * l_noise + LW_COWARD * l_coward
    return np.asarray(total, dtype=np.float32)


# ---------------- execution backends ----------------
def _run_sim(nc, in_maps):
    from concourse.bass_interp import CoreSim
    results = []
    for m in in_maps:
        sim = CoreSim(nc)
        for k, v in m.items():
            sim.tensor(k)[:] = v
        sim.simulate()
        results.append({k: np.array(sim.tensor(k))
                        for k in ("detv_out", "deta_out")})
    return results


def _ensure_ntff_hook():
    """Register the axon NTFF profiling hook if the antenv shim lacks it."""
    import sys
    import types
    try:
        from antenv.axon_hooks import get_axon_ntff_profile_hook  # noqa: F401
        return
    except ImportError:
        pass
    from trn_agent_boot.trn_boot import _ntff_profile_via_ctypes
    hook = _ntff_profile_via_ctypes("/opt/axon/libaxon_pjrt.so")
    mod = types.ModuleType("antenv.axon_hooks")
    _h = [hook]
    mod.set_axon_ntff_profile_hook = lambda h: _h.__setitem__(0, h)
    mod.get_axon_ntff_profile_hook = lambda: _h[0]
    sys.modules["antenv.axon_hooks"] = mod
    import antenv
    antenv.axon_hooks = mod


def _run_hw(nc, in_maps, trace=False):
    import tempfile
    from concourse.bass_utils import run_bass_kernel_spmd
    core_ids = list(range(NCORES))
    if trace:
        try:
            _ensure_ntff_hook()
            tmpdir = tempfile.mkdtemp(prefix="cond_trace_")
            res = run_bass_kernel_spmd(nc, in_maps, core_ids, trace=True,
                                       tmpdir=tmpdir)
            _STATE["last_exec_time_ns"] = res.exec_time_ns
            _STATE["last_trace_dir"] = tmpdir
            _STATE["last_profile_json"] = res.profile_json
            return res.results
        except Exception:
            import traceback
            traceback.print_exc()
            print("[kernel] traced run failed; retrying without trace")
    res = run_bass_kernel_spmd(nc, in_maps, core_ids, trace=False)
    _STATE["last_exec_time_ns"] = res.exec_time_ns
    return res.results


def kernel(beta, x, pt, eta, reconstructable, cluster_ids, n_clusters=None,
           **_ignored):
    in_maps, aux = _prep(beta, x, pt, eta, reconstructable, cluster_ids)
    nc = _get_module()
    if os.environ.get("COND_KERNEL_SIM", "0") == "1":
        results = _run_sim(nc, in_maps)
    else:
        results = _run_hw(nc, in_maps,
                          trace=os.environ.get("COND_KERNEL_TRACE", "0") == "1")
    return _finish(results, aux)


# revision 36
# speedup vs baseline: 1.0029x; 1.0029x over previous
"""Condensation loss (Tiger) on 8 Trainium2 NeuronCores.

Strategy (per sharding hint): shard the hit dimension N across 8 cores,
replicate the K-1 condensation points, assemble the scalar loss on host.

Math restructure vs the baseline kernel: the repulsive term
  v_rep = sum_{n,k} q_n q_k (1 - dist_nk) [dist_nk < 1][~att]
is nonzero only for pairs with d2 < 1.  The device computes the full
N x K d2 matrix on the PE (bf16 inputs, fp32 PSUM) and reduces each row
to a tiny *detector* output instead of evaluating sqrt/min per element:
  - DVE lane:  tensor_reduce(min) -> rowmin of d2
  - ACT lane:  activation(Relu, scale=-1, bias=4) + accum_out
               -> rowsum of relu(4 - d2)
A row can contain a d2 < 1 pair only if its detector fires (bf16 input
rounding shifts d2 by well under the 2.5 flag margin; a guard falls back
to flagging everything for out-of-range inputs).  The host recomputes
flagged rows exactly in fp64 (~1k rows: the condensation points
themselves plus hit 0 for empty objects).  v_att (O(N*D)), l_noise and
l_coward are exact on host in fp64.

Device layout per core: 6400 padded hits = 50 row-tiles of 128.  Each
tile's d2 [128, 1024] lives in one 2-bank PSUM tile; 4 such buffers fill
all 8 banks so each drain engine stays independently double-buffered and
the matmul bursts hide entirely.  Even tiles compute at PE array rows
0:34, odd tiles at 64:98 (tile_position row-packing) so consecutive
tiles' LDWEIGHTS/MATMULs overlap.  Tiles are split ~26/24 between the
DVE and ACT drain lanes (measured ~1.2/1.3 us per [128,1024] fp32 PSUM
drain); both engines run gap-free, which is the 1 elem/cycle/lane fp32
PSUM-read floor of TRN2 (gpsimd and DMA have no PSUM port).
"""

import os
import numpy as np
import ml_dtypes

# ---------------- geometry (hardcoded per the task contract) ----------------
N_HITS = 50000
D_EMB = 32
N_CLUSTERS = 1024          # ids 0..1023; objects are 1..1023
N_OBJ = N_CLUSTERS - 1     # 1023
KP = 1024                  # padded object columns (col j = object j+1; col 1023 dummy)
NCORES = 8
N_PER = N_HITS // NCORES   # 6250
NP = 6400                  # padded rows per core = 50*128
NT = NP // 128             # 50 row tiles
NPAIR = NT // 2            # 25 tile pairs (xt packing unit)
CDIM = D_EMB + 2           # contraction: [x(32), r2, 1]
THR = 4.0                  # detector threshold on d2 (flag margin vs dist<1)
DVE_COST = 1224.0          # ns per tile drain on DVE (measured)
ACT_COST = 1300.0          # ns per tile drain on ACT (measured, incl READ_ACC)

Q_MIN = 0.01
PT_THLD = 0.9
MAX_ETA = 4.0
LW_REP = 1.0
LW_NOISE = 0.1
LW_COWARD = 0.1
EPS = 1e-9

_BF16 = ml_dtypes.bfloat16

_STATE = {}


def _tile_split():
    """Greedy least-loaded assignment of the 50 tile-drains to DVE/ACT."""
    dve, act = [], []
    lv = la = 0.0
    for t in range(NT):
        if lv + DVE_COST <= la + ACT_COST:
            dve.append(t)
            lv += DVE_COST
        else:
            act.append(t)
            la += ACT_COST
    return dve, act


# ---------------- device module ----------------
def _build_module():
    import concourse.bacc as bacc
    import concourse.mybir as mybir
    import concourse.tile as tile
    from contextlib import ExitStack

    dve_tiles, act_tiles = _tile_split()
    nv, na = len(dve_tiles), len(act_tiles)
    lane = {}
    for i, t in enumerate(dve_tiles):
        lane[t] = ("V", i)
    for i, t in enumerate(act_tiles):
        lane[t] = ("A", i)

    nc = bacc.Bacc("TRN2", target_bir_lowering=False, debug=False,
                   num_devices=NCORES)
    dt = mybir.dt

    xt_d = nc.dram_tensor("xt", [128, NPAIR * 128], dt.bfloat16,
                          kind="ExternalInput").ap()
    xkt_d = nc.dram_tensor("xkt", [128, KP], dt.bfloat16,
                           kind="ExternalInput").ap()
    detv_d = nc.dram_tensor("detv_out", [128, nv], dt.float32,
                            kind="ExternalOutput").ap()
    deta_d = nc.dram_tensor("deta_out", [128, na], dt.float32,
                            kind="ExternalOutput").ap()

    with tile.TileContext(nc) as tc, ExitStack() as ctx:
        consts = ctx.enter_context(tc.tile_pool(name="consts", bufs=1))
        scra_p = ctx.enter_context(tc.tile_pool(name="scra", bufs=2))
        psum = ctx.enter_context(tc.tile_pool(name="psum", bufs=4, space="PSUM"))

        # full 128-partition DMAs (partition parallelism sets DMA bandwidth);
        # xkt halves + first xt chunk split across the two DGE queues so the
        # first matmuls' operands land as early as possible
        xkt_sb = consts.tile([128, KP], dt.bfloat16)
        xt_sb = consts.tile([128, NPAIR * 128], dt.bfloat16)
        nc.scalar.dma_start(out=xkt_sb[:, 0:512], in_=xkt_d[:, 0:512])
        nc.sync.dma_start(out=xt_sb[:, 0:256], in_=xt_d[:, 0:256])
        nc.sync.dma_start(out=xkt_sb[:, 512:1024], in_=xkt_d[:, 512:1024])
        edges = [256, 896, 1664, 2432, NPAIR * 128]
        for a, b in zip(edges[:-1], edges[1:]):
            nc.sync.dma_start(out=xt_sb[:, a:b], in_=xt_d[:, a:b])
        thrb_sb = consts.tile([128, 1], dt.float32)
        nc.gpsimd.memset(thrb_sb, THR)
        detv_sb = consts.tile([128, nv], dt.float32)
        deta_sb = consts.tile([128, na], dt.float32)

        # PE warmup: ~3.4us of back-to-back dummy matmuls during the DMA
        # wait flips the HAM clock gate to 2.4 GHz before the real work
        zw_sb = consts.tile([128, 512], dt.bfloat16)
        nc.gpsimd.memset(zw_sb, 0.0)
        ps_w = psum.tile([128, 1024], dt.float32, tag="d2")
        for i in range(8):
            half = slice(0, 512) if i % 2 == 0 else slice(512, 1024)
            nc.tensor.matmul(ps_w[:, half], zw_sb[0:CDIM, 0:128],
                             zw_sb[0:CDIM, 0:512],
                             start=True, stop=True, tile_position=(0, 0))

        for t in range(NT):
            # even tiles live at PE array rows 0:34, odd tiles at 64:98 —
            # consecutive tiles' LDWEIGHTS/MATMULs overlap (per-subarray
            # concurrency), and the 4 psum buffers keep both drain engines
            # independently double-buffered.
            p, base = t // 2, (0 if t % 2 == 0 else 64)
            ps = psum.tile([128, 1024], dt.float32, tag="d2")
            lhs = xt_sb[base:base + CDIM, p * 128:(p + 1) * 128]
            nc.tensor.matmul(ps[:, 0:512], lhs, xkt_sb[base:base + CDIM, 0:512],
                             start=True, stop=True, tile_position=(base, 0))
            nc.tensor.matmul(ps[:, 512:1024], lhs,
                             xkt_sb[base:base + CDIM, 512:1024],
                             start=True, stop=True, tile_position=(base, 0))
            which, idx = lane[t]
            if which == "V":
                nc.vector.tensor_reduce(detv_sb[:, idx:idx + 1], ps,
                                        axis=mybir.AxisListType.X,
                                        op=mybir.AluOpType.min)
            else:
                scr = scra_p.tile([128, 1024], dt.bfloat16, tag="scra")
                nc.scalar.activation(
                    scr, ps, mybir.ActivationFunctionType.Relu,
                    bias=thrb_sb, scale=-1.0,
                    accum_out=deta_sb[:, idx:idx + 1])

        # drain the finished halves of the det outputs early; final halves
        # go out on both DGE queues in parallel
        nc.sync.dma_start(out=detv_d[:, 0:nv // 2], in_=detv_sb[:, 0:nv // 2])
        nc.scalar.dma_start(out=deta_d[:, 0:na // 2], in_=deta_sb[:, 0:na // 2])
        nc.sync.dma_start(out=detv_d[:, nv // 2:], in_=detv_sb[:, nv // 2:])
        nc.scalar.dma_start(out=deta_d[:, na // 2:], in_=deta_sb[:, na // 2:])

    nc.compile()
    return nc


def _get_module():
    if "nc" not in _STATE:
        _STATE["nc"] = _build_module()
    return _STATE["nc"]


# ---------------- host prep ----------------
def _prep(beta, x, pt, eta, reconstructable, cluster_ids):
    f32 = np.float32
    f64 = np.float64
    beta = np.asarray(beta, f32)
    x = np.ascontiguousarray(np.asarray(x, f32))
    pt = np.asarray(pt, f32)
    eta = np.asarray(eta, f32)
    recon = np.asarray(reconstructable)
    cid = np.asarray(cluster_ids).astype(np.int64)

    # alpha selection in fp32 to match the reference's argmax semantics
    q32 = (np.arctanh(np.clip(beta, 0.0, 1.0 - 1e-4)) ** 2 + Q_MIN).astype(f32)
    hit_ok = (recon > 0) & (pt > PT_THLD) & (np.abs(eta) < MAX_ETA)
    cid_eff = np.where(hit_ok, cid, 0)
    best = np.zeros(N_CLUSTERS, f32)
    np.maximum.at(best, cid_eff, q32)
    idx = np.full(N_CLUSTERS, N_HITS, np.int64)
    ismax = (q32 == best[cid_eff]) & (cid_eff > 0)
    np.minimum.at(idx, cid_eff[ismax], np.nonzero(ismax)[0])
    alphas = np.where(idx[1:] < N_HITS, idx[1:], 0)      # [1023]

    # device operands: bf16-quantized hits + condensation points
    xq = x.astype(_BF16)                                 # [N, 32]
    xqf = xq.astype(f32)
    r2q = np.einsum('nd,nd->n', xqf, xqf).astype(f32)
    r2b = r2q.astype(_BF16)

    X34 = np.zeros((NCORES * NP, CDIM), f32)
    real = np.zeros(NCORES * NP, bool)
    for c in range(NCORES):
        real[c * NP:c * NP + N_PER] = True
    X34[real, :D_EMB] = xqf
    X34[real, D_EMB] = r2b.astype(f32)
    X34[:, D_EMB + 1] = 1.0
    X34 = X34.astype(_BF16)

    Y34 = np.zeros((KP, CDIM), f32)
    Y34[:N_OBJ, :D_EMB] = -2.0 * xqf[alphas]
    Y34[:N_OBJ, D_EMB] = 1.0
    Y34[:N_OBJ, D_EMB + 1] = r2b[alphas].astype(f32)
    Y34[N_OBJ] = 0.0
    Y34[N_OBJ, D_EMB] = 1.0
    Y34[N_OBJ, D_EMB + 1] = 1e4                          # dummy far column
    Y34 = Y34.astype(_BF16)
    xkt = np.zeros((128, KP), _BF16)
    xkt[0:CDIM] = Y34.T
    xkt[64:64 + CDIM] = Y34.T

    in_maps = []
    for c in range(NCORES):
        A = X34[c * NP:(c + 1) * NP].reshape(NT, 128, CDIM).transpose(0, 2, 1)
        xt_c = np.zeros((128, NPAIR * 128), _BF16)
        xt_c[0:CDIM] = A[0::2].transpose(1, 0, 2).reshape(CDIM, NPAIR * 128)
        xt_c[64:64 + CDIM] = A[1::2].transpose(1, 0, 2).reshape(
            CDIM, NPAIR * 128)
        in_maps.append({"xt": np.ascontiguousarray(xt_c), "xkt": xkt})

    aux = dict(q32=q32, hit_ok=hit_ok, cid=cid, beta=beta, x=x,
               alphas=alphas)
    return in_maps, aux


# ---------------- host finish ----------------
def _finish(results, aux):
    f64 = np.float64
    q32, alphas = aux["q32"], aux["alphas"]
    hit_ok, cid, beta, x = aux["hit_ok"], aux["cid"], aux["beta"], aux["x"]

    q = q32.astype(f64)
    x64 = x.astype(f64)
    xk64 = x64[alphas]                                   # [1023, 32]
    qk = q[alphas]

    dve_tiles, act_tiles = _tile_split()

    # ---- gather flagged hits from the detectors ----
    flagged = set()
    for c in range(NCORES):
        detv = np.asarray(results[c]["detv_out"], f64)   # [128, nv]
        deta = np.asarray(results[c]["deta_out"], f64)   # [128, na]
        fl = np.zeros((128, NT), bool)
        fl[:, dve_tiles] = detv < THR - 0.5
        fl[:, act_tiles] = deta > 0.45
        rr, tt = np.nonzero(fl)
        for r, t in zip(rr, tt):
            n = t * 128 + r
            if n < N_PER:
                flagged.add(c * N_PER + n)
    flagged = np.fromiter(sorted(flagged), dtype=np.int64,
                          count=len(flagged))

    # safety: the detector's bf16 error margin assumes moderate |x|; the
    # dominant term is the bf16 rounding of |x|^2, so bound that directly
    if (not np.isfinite(x).all()) or \
            float(np.einsum('nd,nd->n', x64, x64).max()) > 200.0:
        flagged = np.arange(N_HITS, dtype=np.int64)
    if os.environ.get("COND_KERNEL_DEBUG", "0") == "1":
        print(f"[kernel] flagged rows: {len(flagged)}")

    # ---- v_rep: exact fp64 over flagged rows only ----
    v_rep_num = 0.0
    if len(flagged):
        xf = x64[flagged]
        d2 = (np.einsum('nd,nd->n', xf, xf)[:, None]
              + np.einsum('kd,kd->k', xk64, xk64)[None, :]
              - 2.0 * (xf @ xk64.T))
        dist = np.sqrt(np.maximum(d2, 1e-12))
        att = (cid[flagged][:, None] == np.arange(1, N_CLUSTERS)[None, :]) \
            & hit_ok[flagged][:, None]
        rep = (~att) & (dist < 1.0)
        qw = q[flagged][:, None] * qk[None, :]
        v_rep_num = float(np.sum(qw * (1.0 - dist) * rep))

    # ---- v_att: exact fp64 on the attractive pairs ----
    att_idx = np.nonzero(hit_ok & (cid > 0))[0]
    kk = cid[att_idx] - 1
    diff = x64[att_idx] - xk64[kk]
    d2a = np.maximum(np.einsum('nd,nd->n', diff, diff), 1e-12)
    v_att_num = float(np.sum(q[att_idx] * qk[kk] * d2a))

    n_hits_oi = float(hit_ok.sum())
    norm_att = EPS + n_hits_oi - N_OBJ
    norm_rep = EPS + (N_OBJ - 1) * N_HITS
    v_att = v_att_num / norm_att
    v_rep = v_rep_num / norm_rep

    noise_mask = (cid <= 0)
    l_noise = float(beta[noise_mask].sum()) / max(float(noise_mask.sum()), 1.0)
    l_coward = float(np.mean(1.0 - beta[alphas]))

    total = v_att + LW_REP * v_rep + LW_NOISE * l_noise + LW_COWARD * l_coward
    return np.asarray(total, dtype=np.float32)


# ---------------- execution backends ----------------
def _run_sim(nc, in_maps):
    from concourse.bass_interp import CoreSim
    results = []
    for m in in_maps:
        sim = CoreSim(nc)
        for k, v in m.items():
            sim.tensor(k)[:] = v
        sim.simulate()
        results.append({k: np.array(sim.tensor(k))
                        for k in ("detv_out", "deta_out")})
    return results


def _ensure_ntff_hook():
    """Register the axon NTFF profiling hook if the antenv shim lacks it."""
    import sys
    import types
    try:
        from antenv.axon_hooks import get_axon_ntff_profile_hook  # noqa: F401
        return
    except ImportError:
        pass
    from trn_agent_boot.trn_boot import _ntff_profile_via_ctypes
    hook = _ntff_profile_via_ctypes("/opt/axon/libaxon_pjrt.so")
    mod = types.ModuleType("antenv.axon_hooks")
    _h = [hook]
    mod.set_axon_ntff_profile_hook = lambda h: _h.__setitem__(0, h)
    mod.get_axon_ntff_profile_hook = lambda: _h[0]
    sys.modules["antenv.axon_hooks"] = mod
    import antenv
    antenv.axon_hooks = mod


def _run_hw(nc, in_maps, trace=False):
    import tempfile
    from concourse.bass_utils import run_bass_kernel_spmd
    core_ids = list(range(NCORES))
    if trace:
        try:
            _ensure_ntff_hook()
            tmpdir = tempfile.mkdtemp(prefix="cond_trace_")
            res = run_bass_kernel_spmd(nc, in_maps, core_ids, trace=True,
                                       tmpdir=tmpdir)
            _STATE["last_exec_time_ns"] = res.exec_time_ns
            _STATE["last_trace_dir"] = tmpdir
            _STATE["last_profile_json"] = res.profile_json
            return res.results
        except Exception:
            import traceback
            traceback.print_exc()
            print("[kernel] traced run failed; retrying without trace")
    res = run_bass_kernel_spmd(nc, in_maps, core_ids, trace=False)
    _STATE["last_exec_time_ns"] = res.exec_time_ns
    return res.results


def kernel(beta, x, pt, eta, reconstructable, cluster_ids, n_clusters=None,
           **_ignored):
    in_maps, aux = _prep(beta, x, pt, eta, reconstructable, cluster_ids)
    nc = _get_module()
    if os.environ.get("COND_KERNEL_SIM", "0") == "1":
        results = _run_sim(nc, in_maps)
    else:
        results = _run_hw(nc, in_maps,
                          trace=os.environ.get("COND_KERNEL_TRACE", "0") == "1")
    return _finish(results, aux)


# revision 37
# speedup vs baseline: 1.0078x; 1.0048x over previous
"""Condensation loss (Tiger) on 8 Trainium2 NeuronCores.

Strategy (per sharding hint): shard the hit dimension N across 8 cores,
replicate the K-1 condensation points, assemble the scalar loss on host.

Math restructure vs the baseline kernel: the repulsive term
  v_rep = sum_{n,k} q_n q_k (1 - dist_nk) [dist_nk < 1][~att]
is nonzero only for pairs with d2 < 1.  The device computes the full
N x K d2 matrix on the PE (bf16 inputs, fp32 PSUM) and reduces each row
to a tiny *detector* output instead of evaluating sqrt/min per element:
  - DVE lane:  tensor_reduce(min) -> rowmin of d2
  - ACT lane:  activation(Relu, scale=-1, bias=4) + accum_out
               -> rowsum of relu(4 - d2)
A row can contain a d2 < 1 pair only if its detector fires (bf16 input
rounding shifts d2 by well under the 2.5 flag margin; a guard falls back
to flagging everything for out-of-range inputs).  The host recomputes
flagged rows exactly in fp64 (~1k rows: the condensation points
themselves plus hit 0 for empty objects).  v_att (O(N*D)), l_noise and
l_coward are exact on host in fp64.

Device layout per core: 6400 padded hits = 50 row-tiles of 128.  Each
tile's d2 [128, 1024] lives in one 2-bank PSUM tile; 4 such buffers fill
all 8 banks so each drain engine stays independently double-buffered and
the matmul bursts hide entirely.  Even tiles compute at PE array rows
0:34, odd tiles at 64:98 (tile_position row-packing) so consecutive
tiles' LDWEIGHTS/MATMULs overlap.  Tiles are split ~26/24 between the
DVE and ACT drain lanes (measured ~1.2/1.3 us per [128,1024] fp32 PSUM
drain); both engines run gap-free, which is the 1 elem/cycle/lane fp32
PSUM-read floor of TRN2 (gpsimd and DMA have no PSUM port).
"""

import os
import numpy as np
import ml_dtypes

# ---------------- geometry (hardcoded per the task contract) ----------------
N_HITS = 50000
D_EMB = 32
N_CLUSTERS = 1024          # ids 0..1023; objects are 1..1023
N_OBJ = N_CLUSTERS - 1     # 1023
KP = 1024                  # padded object columns (col j = object j+1; col 1023 dummy)
NCORES = 8
N_PER = N_HITS // NCORES   # 6250
NP = 6400                  # padded rows per core = 50*128
NT = NP // 128             # 50 row tiles
NPAIR = NT // 2            # 25 tile pairs (xt packing unit)
CDIM = D_EMB + 2           # contraction: [x(32), r2, 1]
THR = 4.0                  # detector threshold on d2 (flag margin vs dist<1)
DVE_COST = 1224.0          # ns per tile drain on DVE (measured)
ACT_COST = 1300.0          # ns per tile drain on ACT (measured, incl READ_ACC)

Q_MIN = 0.01
PT_THLD = 0.9
MAX_ETA = 4.0
LW_REP = 1.0
LW_NOISE = 0.1
LW_COWARD = 0.1
EPS = 1e-9

_BF16 = ml_dtypes.bfloat16

_STATE = {}


def _tile_split():
    """Greedy least-loaded assignment of the 50 tile-drains to DVE/ACT."""
    dve, act = [], []
    lv = la = 0.0
    for t in range(NT):
        if lv + DVE_COST <= la + ACT_COST:
            dve.append(t)
            lv += DVE_COST
        else:
            act.append(t)
            la += ACT_COST
    return dve, act


# ---------------- device module ----------------
def _build_module():
    import concourse.bacc as bacc
    import concourse.mybir as mybir
    import concourse.tile as tile
    from contextlib import ExitStack

    dve_tiles, act_tiles = _tile_split()
    nv, na = len(dve_tiles), len(act_tiles)
    lane = {}
    for i, t in enumerate(dve_tiles):
        lane[t] = ("V", i)
    for i, t in enumerate(act_tiles):
        lane[t] = ("A", i)

    nc = bacc.Bacc("TRN2", target_bir_lowering=False, debug=False,
                   num_devices=NCORES)
    dt = mybir.dt

    xt_d = nc.dram_tensor("xt", [128, NPAIR * 128], dt.bfloat16,
                          kind="ExternalInput").ap()
    xkt_d = nc.dram_tensor("xkt", [128, KP], dt.bfloat16,
                           kind="ExternalInput").ap()
    detv_d = nc.dram_tensor("detv_out", [128, nv], dt.float32,
                            kind="ExternalOutput").ap()
    deta_d = nc.dram_tensor("deta_out", [128, na], dt.float32,
                            kind="ExternalOutput").ap()

    with tile.TileContext(nc) as tc, ExitStack() as ctx:
        consts = ctx.enter_context(tc.tile_pool(name="consts", bufs=1))
        scra_p = ctx.enter_context(tc.tile_pool(name="scra", bufs=2))
        psum = ctx.enter_context(tc.tile_pool(name="psum", bufs=4, space="PSUM"))

        # full 128-partition DMAs (partition parallelism sets DMA bandwidth);
        # xkt halves + first xt chunk split across the two DGE queues so the
        # first matmuls' operands land as early as possible
        xkt_sb = consts.tile([128, KP], dt.bfloat16)
        xt_sb = consts.tile([128, NPAIR * 128], dt.bfloat16)
        nc.scalar.dma_start(out=xkt_sb[:, 0:512], in_=xkt_d[:, 0:512])
        nc.sync.dma_start(out=xt_sb[:, 0:256], in_=xt_d[:, 0:256])
        nc.sync.dma_start(out=xkt_sb[:, 512:1024], in_=xkt_d[:, 512:1024])
        edges = [256, 896, 1664, 2432, NPAIR * 128]
        for a, b in zip(edges[:-1], edges[1:]):
            nc.sync.dma_start(out=xt_sb[:, a:b], in_=xt_d[:, a:b])
        thrb_sb = consts.tile([128, 1], dt.float32)
        nc.gpsimd.memset(thrb_sb, THR)
        detv_sb = consts.tile([128, nv], dt.float32)
        deta_sb = consts.tile([128, na], dt.float32)

        for t in range(NT):
            # even tiles live at PE array rows 0:34, odd tiles at 64:98 —
            # consecutive tiles' LDWEIGHTS/MATMULs overlap (per-subarray
            # concurrency), and the 4 psum buffers keep both drain engines
            # independently double-buffered.
            p, base = t // 2, (0 if t % 2 == 0 else 64)
            ps = psum.tile([128, 1024], dt.float32, tag="d2")
            lhs = xt_sb[base:base + CDIM, p * 128:(p + 1) * 128]
            nc.tensor.matmul(ps[:, 0:512], lhs, xkt_sb[base:base + CDIM, 0:512],
                             start=True, stop=True, tile_position=(base, 0))
            nc.tensor.matmul(ps[:, 512:1024], lhs,
                             xkt_sb[base:base + CDIM, 512:1024],
                             start=True, stop=True, tile_position=(base, 0))
            which, idx = lane[t]
            if which == "V":
                nc.vector.tensor_reduce(detv_sb[:, idx:idx + 1], ps,
                                        axis=mybir.AxisListType.X,
                                        op=mybir.AluOpType.min)
            else:
                scr = scra_p.tile([128, 1024], dt.bfloat16, tag="scra")
                nc.scalar.activation(
                    scr, ps, mybir.ActivationFunctionType.Relu,
                    bias=thrb_sb, scale=-1.0,
                    accum_out=deta_sb[:, idx:idx + 1])

        # drain the finished halves of the det outputs early; final halves
        # go out on both DGE queues in parallel
        nc.sync.dma_start(out=detv_d[:, 0:nv // 2], in_=detv_sb[:, 0:nv // 2])
        nc.scalar.dma_start(out=deta_d[:, 0:na // 2], in_=deta_sb[:, 0:na // 2])
        nc.sync.dma_start(out=detv_d[:, nv // 2:], in_=detv_sb[:, nv // 2:])
        nc.scalar.dma_start(out=deta_d[:, na // 2:], in_=deta_sb[:, na // 2:])

    nc.compile()
    return nc


def _get_module():
    if "nc" not in _STATE:
        _STATE["nc"] = _build_module()
    return _STATE["nc"]


# ---------------- host prep ----------------
def _prep(beta, x, pt, eta, reconstructable, cluster_ids):
    f32 = np.float32
    f64 = np.float64
    beta = np.asarray(beta, f32)
    x = np.ascontiguousarray(np.asarray(x, f32))
    pt = np.asarray(pt, f32)
    eta = np.asarray(eta, f32)
    recon = np.asarray(reconstructable)
    cid = np.asarray(cluster_ids).astype(np.int64)

    # alpha selection in fp32 to match the reference's argmax semantics
    q32 = (np.arctanh(np.clip(beta, 0.0, 1.0 - 1e-4)) ** 2 + Q_MIN).astype(f32)
    hit_ok = (recon > 0) & (pt > PT_THLD) & (np.abs(eta) < MAX_ETA)
    cid_eff = np.where(hit_ok, cid, 0)
    best = np.zeros(N_CLUSTERS, f32)
    np.maximum.at(best, cid_eff, q32)
    idx = np.full(N_CLUSTERS, N_HITS, np.int64)
    ismax = (q32 == best[cid_eff]) & (cid_eff > 0)
    np.minimum.at(idx, cid_eff[ismax], np.nonzero(ismax)[0])
    alphas = np.where(idx[1:] < N_HITS, idx[1:], 0)      # [1023]

    # device operands: bf16-quantized hits + condensation points
    xq = x.astype(_BF16)                                 # [N, 32]
    xqf = xq.astype(f32)
    r2q = np.einsum('nd,nd->n', xqf, xqf).astype(f32)
    r2b = r2q.astype(_BF16)

    X34 = np.zeros((NCORES * NP, CDIM), f32)
    real = np.zeros(NCORES * NP, bool)
    for c in range(NCORES):
        real[c * NP:c * NP + N_PER] = True
    X34[real, :D_EMB] = xqf
    X34[real, D_EMB] = r2b.astype(f32)
    X34[:, D_EMB + 1] = 1.0
    X34 = X34.astype(_BF16)

    Y34 = np.zeros((KP, CDIM), f32)
    Y34[:N_OBJ, :D_EMB] = -2.0 * xqf[alphas]
    Y34[:N_OBJ, D_EMB] = 1.0
    Y34[:N_OBJ, D_EMB + 1] = r2b[alphas].astype(f32)
    Y34[N_OBJ] = 0.0
    Y34[N_OBJ, D_EMB] = 1.0
    Y34[N_OBJ, D_EMB + 1] = 1e4                          # dummy far column
    Y34 = Y34.astype(_BF16)
    xkt = np.zeros((128, KP), _BF16)
    xkt[0:CDIM] = Y34.T
    xkt[64:64 + CDIM] = Y34.T

    in_maps = []
    for c in range(NCORES):
        A = X34[c * NP:(c + 1) * NP].reshape(NT, 128, CDIM).transpose(0, 2, 1)
        xt_c = np.zeros((128, NPAIR * 128), _BF16)
        xt_c[0:CDIM] = A[0::2].transpose(1, 0, 2).reshape(CDIM, NPAIR * 128)
        xt_c[64:64 + CDIM] = A[1::2].transpose(1, 0, 2).reshape(
            CDIM, NPAIR * 128)
        in_maps.append({"xt": np.ascontiguousarray(xt_c), "xkt": xkt})

    aux = dict(q32=q32, hit_ok=hit_ok, cid=cid, beta=beta, x=x,
               alphas=alphas)
    return in_maps, aux


# ---------------- host finish ----------------
def _finish(results, aux):
    f64 = np.float64
    q32, alphas = aux["q32"], aux["alphas"]
    hit_ok, cid, beta, x = aux["hit_ok"], aux["cid"], aux["beta"], aux["x"]

    q = q32.astype(f64)
    x64 = x.astype(f64)
    xk64 = x64[alphas]                                   # [1023, 32]
    qk = q[alphas]

    dve_tiles, act_tiles = _tile_split()

    # ---- gather flagged hits from the detectors ----
    flagged = set()
    for c in range(NCORES):
        detv = np.asarray(results[c]["detv_out"], f64)   # [128, nv]
        deta = np.asarray(results[c]["deta_out"], f64)   # [128, na]
        fl = np.zeros((128, NT), bool)
        fl[:, dve_tiles] = detv < THR - 0.5
        fl[:, act_tiles] = deta > 0.45
        rr, tt = np.nonzero(fl)
        for r, t in zip(rr, tt):
            n = t * 128 + r
            if n < N_PER:
                flagged.add(c * N_PER + n)
    flagged = np.fromiter(sorted(flagged), dtype=np.int64,
                          count=len(flagged))

    # safety: the detector's bf16 error margin assumes moderate |x|; the
    # dominant term is the bf16 rounding of |x|^2, so bound that directly
    if (not np.isfinite(x).all()) or \
            float(np.einsum('nd,nd->n', x64, x64).max()) > 200.0:
        flagged = np.arange(N_HITS, dtype=np.int64)
    if os.environ.get("COND_KERNEL_DEBUG", "0") == "1":
        print(f"[kernel] flagged rows: {len(flagged)}")

    # ---- v_rep: exact fp64 over flagged rows only ----
    v_rep_num = 0.0
    if len(flagged):
        xf = x64[flagged]
        d2 = (np.einsum('nd,nd->n', xf, xf)[:, None]
              + np.einsum('kd,kd->k', xk64, xk64)[None, :]
              - 2.0 * (xf @ xk64.T))
        dist = np.sqrt(np.maximum(d2, 1e-12))
        att = (cid[flagged][:, None] == np.arange(1, N_CLUSTERS)[None, :]) \
            & hit_ok[flagged][:, None]
        rep = (~att) & (dist < 1.0)
        qw = q[flagged][:, None] * qk[None, :]
        v_rep_num = float(np.sum(qw * (1.0 - dist) * rep))

    # ---- v_att: exact fp64 on the attractive pairs ----
    att_idx = np.nonzero(hit_ok & (cid > 0))[0]
    kk = cid[att_idx] - 1
    diff = x64[att_idx] - xk64[kk]
    d2a = np.maximum(np.einsum('nd,nd->n', diff, diff), 1e-12)
    v_att_num = float(np.sum(q[att_idx] * qk[kk] * d2a))

    n_hits_oi = float(hit_ok.sum())
    norm_att = EPS + n_hits_oi - N_OBJ
    norm_rep = EPS + (N_OBJ - 1) * N_HITS
    v_att = v_att_num / norm_att
    v_rep = v_rep_num / norm_rep

    noise_mask = (cid <= 0)
    l_noise = float(beta[noise_mask].sum()) / max(float(noise_mask.sum()), 1.0)
    l_coward = float(np.mean(1.0 - beta[alphas]))

    total = v_att + LW_REP * v_rep + LW_NOISE * l_noise + LW_COWARD * l_coward
    return np.asarray(total, dtype=np.float32)


# ---------------- execution backends ----------------
def _run_sim(nc, in_maps):
    from concourse.bass_interp import CoreSim
    results = []
    for m in in_maps:
        sim = CoreSim(nc)
        for k, v in m.items():
            sim.tensor(k)[:] = v
        sim.simulate()
        results.append({k: np.array(sim.tensor(k))
                        for k in ("detv_out", "deta_out")})
    return results


def _ensure_ntff_hook():
    """Register the axon NTFF profiling hook if the antenv shim lacks it."""
    import sys
    import types
    try:
        from antenv.axon_hooks import get_axon_ntff_profile_hook  # noqa: F401
        return
    except ImportError:
        pass
    from trn_agent_boot.trn_boot import _ntff_profile_via_ctypes
    hook = _ntff_profile_via_ctypes("/opt/axon/libaxon_pjrt.so")
    mod = types.ModuleType("antenv.axon_hooks")
    _h = [hook]
    mod.set_axon_ntff_profile_hook = lambda h: _h.__setitem__(0, h)
    mod.get_axon_ntff_profile_hook = lambda: _h[0]
    sys.modules["antenv.axon_hooks"] = mod
    import antenv
    antenv.axon_hooks = mod


def _run_hw(nc, in_maps, trace=False):
    import tempfile
    from concourse.bass_utils import run_bass_kernel_spmd
    core_ids = list(range(NCORES))
    if trace:
        try:
            _ensure_ntff_hook()
            tmpdir = tempfile.mkdtemp(prefix="cond_trace_")
            res = run_bass_kernel_spmd(nc, in_maps, core_ids, trace=True,
                                       tmpdir=tmpdir)
            _STATE["last_exec_time_ns"] = res.exec_time_ns
            _STATE["last_trace_dir"] = tmpdir
            _STATE["last_profile_json"] = res.profile_json
            return res.results
        except Exception:
            import traceback
            traceback.print_exc()
            print("[kernel] traced run failed; retrying without trace")
    res = run_bass_kernel_spmd(nc, in_maps, core_ids, trace=False)
    _STATE["last_exec_time_ns"] = res.exec_time_ns
    return res.results


def kernel(beta, x, pt, eta, reconstructable, cluster_ids, n_clusters=None,
           **_ignored):
    in_maps, aux = _prep(beta, x, pt, eta, reconstructable, cluster_ids)
    nc = _get_module()
    if os.environ.get("COND_KERNEL_SIM", "0") == "1":
        results = _run_sim(nc, in_maps)
    else:
        results = _run_hw(nc, in_maps,
                          trace=os.environ.get("COND_KERNEL_TRACE", "0") == "1")
    return _finish(results, aux)
